# revision 1
# baseline (speedup 1.0000x reference)
"""FourierKAN layer (N=16384, I=128, O=128, G=16) on 8 Trainium2 NeuronCores.

out[n,o] = LN_o( sum_{i,g} cos(g*x[n,i])*Ac[o,i,g] + sin(g*x[n,i])*As[o,i,g]
                 + bias[o] ) * gamma + beta

Strategy (data-parallel over N, 2048 rows/core):
 - Device computes a basis of 37 [I=128, n] fp16 tiles whose span (as
   functions of x) covers all 32 harmonics {cos(gx), sin(gx), g=1..16} + const:
     * direct pairs g in {1,3}: fp32 range-reduction (rint via +1.5*2^23
       trick) then ACT Sin; cos via Square(Sin(pi*t)) identity.
     * doubling chain 2,4,6,8,12,16: ACT Square(c_g) + DVE c_g*s_g.
     * product quads (a,b): 4 elementwise products span cos/sin((a+-b)x).
 - Host solves exact least-squares weights W[b,i,o] (fp64) mapping basis ->
   amplitudes, centers over o (folds LayerNorm mean-subtraction into weights).
 - GEMM: 37 accumulating fp16 matmuls per 512-col tile -> y_c [O,n] PSUM.
 - LN: Sigma y_c^2 via ones-matmul; rstd = exp(-0.5*ln(var+eps)) on ACT;
   broadcast rstd via K=1 matmul; final scale on DVE + gamma/beta on ACT.
 - Output is produced [O, N]-layout on device; host transposes to (N, O).
"""
import sys

sys.path.insert(0, "/opt/trn_rl_repo")

import numpy as np

import concourse.bass as bass
import concourse.mybir as mybir
from concourse.tile import TileContext
from contextlib import ExitStack

# ---------------------------------------------------------------------------
# walrus in this container accepts at most ONE sync-wait command per
# instruction; TileContext's tail drain and ordinary joins can carry more.
# Patch: split waits onto same-engine InstNoOp carriers.
# ---------------------------------------------------------------------------
import bass_rust
from concourse import tile as _tile


def _patched_drain_and_barrier(self, tick_clock, wait_clock):
    nc = self.nc
    gc = tick_clock.global_clock
    n = len(gc)
    for p in range(n):
        if gc[p] > 0:
            vc = bass_rust.VectorClock([0] * n)
            vc.require_at_least(p, gc[p])
            nop = nc.sync.nop(hint="drain_wait_carrier", nofuse=True)
            wait_clock.add_sem_waits(nop.ins, bass_rust.ScopedClock({None: vc}))
    nc.sync.drain()
    nc.all_engine_barrier()
    assert self.sems is not None
    popped = nc._tile_sem_poison_stack.pop()
    assert popped is self._sem_poison
    nc.clear_and_free_semaphores(list(self.sems.allocated().values()))
    nc.all_engine_barrier()


_orig_lower = _tile.TileContext._lower_ordered_insts


def _patched_lower_ordered_insts(self, ordered):
    for bb_name, insts in ordered.items():
        new = []
        for inst in insts:
            si = getattr(inst, "sync_info", None)
            eng = getattr(inst, "engine", None)
            if (
                si is not None
                and si.on_wait
                and len(si.on_wait) > 1
                and eng is not None
                and isinstance(inst, mybir.Instruction)
            ):
                waits = list(si.on_wait)
                for w in waits[:-1]:
                    new.append(
                        mybir.InstNoOp(
                            name=self.nc.get_next_instruction_name(),
                            sync_info=mybir.SyncInfo(on_wait=[w], on_update=[]),
                            bass_nofuse=True,
                            engine=eng,
                        )
                    )
                inst.sync_info = mybir.SyncInfo(
                    on_wait=[waits[-1]], on_update=list(si.on_update)
                )
            new.append(inst)
        insts[:] = new
    return _orig_lower(self, ordered)


_tile.TileContext._drain_and_barrier = _patched_drain_and_barrier
_tile.TileContext._lower_ordered_insts = _patched_lower_ordered_insts

# ---------------------------------------------------------------------------
# Problem constants
# ---------------------------------------------------------------------------
N, I, O, G = 16384, 128, 128, 16
NCORES = 8
NSH = N // NCORES  # 2048 rows per core
JT = 512  # GEMM moving-tile width
NJ = NSH // JT  # 4
F32 = mybir.dt.float32
F16 = mybir.dt.float16
A = mybir.AluOpType
AF = mybir.ActivationFunctionType
TWO_PI = 2.0 * np.pi
RC = 12582912.0  # 1.5 * 2^23 : fp32 round-to-int magic constant
EPS = 1e-5

# ---------------------------------------------------------------------------
# Basis specification. Each op appends basis/mat tiles with an exact harmonic
# expansion dict {('c',g)|('s',g)|('1',0): coef}. Tiles are named; device ops
# are emitted from the same spec so host and device agree exactly.
# ---------------------------------------------------------------------------


def _expand_mul(e1, e2):
    out = {}

    def add(k, v):
        if abs(v) > 1e-15:
            out[k] = out.get(k, 0.0) + v

    for (k1, g1), v1 in e1.items():
        for (k2, g2), v2 in e2.items():
            v = v1 * v2
            if k1 == "1" and k2 == "1":
                add(("1", 0), v)
            elif k1 == "1":
                add((k2, g2), v)
            elif k2 == "1":
                add((k1, g1), v)
            elif k1 == "c" and k2 == "c":
                add(_n("c", g1 + g2), 0.5 * v)
                add(_n("c", g1 - g2), 0.5 * v)
            elif k1 == "s" and k2 == "s":
                add(_n("c", g1 - g2), 0.5 * v)
                add(_n("c", g1 + g2), -0.5 * v)
            elif k1 == "s" and k2 == "c":
                add(_n("s", g1 + g2), 0.5 * v)
                add(_n("s", g1 - g2), 0.5 * v)
            else:  # c * s
                add(_n("s", g1 + g2), 0.5 * v)
                add(_n("s", g1 - g2), -0.5 * v)
    # canonicalize negatives / zeros
    res = {}
    for (k, g), v in out.items():
        if abs(v) > 1e-15:
            res[(k, g)] = res.get((k, g), 0.0) + v
    return {k: v for k, v in res.items() if abs(v) > 1e-15}


def _n(kind, g):
    # canonical harmonic key: cos(-g)=cos(g); sin(-g)=-sin(g); sin(0)=0->dropped
    if g < 0:
        if kind == "c":
            return ("c", -g)
        return ("s_neg", -g)  # handled below
    if g == 0:
        if kind == "c":
            return ("1", 0)
        return ("zero", 0)
    return (kind, g)


def _expand_mul_fix(e1, e2):
    raw = _expand_mul(e1, e2)
    out = {}
    for (k, g), v in raw.items():
        if k == "s_neg":
            out[("s", g)] = out.get(("s", g), 0.0) - v
        elif k == "zero":
            pass
        else:
            out[(k, g)] = out.get((k, g), 0.0) + v
    return {k: v for k, v in out.items() if abs(v) > 1e-15}


def _affine(e, a, b):
    out = {k: a * v for k, v in e.items()}
    out[("1", 0)] = out.get(("1", 0), 0.0) + b
    return {k: v for k, v in out.items() if abs(v) > 1e-15}


class Spec:
    """Device program spec: list of ops over named fp16 tiles + expansions."""

    def __init__(self):
        self.exp = {}  # name -> expansion dict
        self.ops = []  # (kind, args...)
        self.basis = []  # names feeding the GEMM, in order

    # direct pair via range reduction; tiles: s{g} (basis), q{g} (basis),
    # optionally c{g} materialized pure for use as a product carrier.
    def direct(self, g, mat_c=False):
        self.ops.append(("frac", g))  # produces fp32 t_g
        sn, shn, qn = f"s{g}", f"sh{g}", f"q{g}"
        self.ops.append(("sin2pi", g, sn))
        self.exp[sn] = {("s", g): 1.0}
        self.ops.append(("sinpi", g, shn))
        self.ops.append(("square", shn, qn))
        self.exp[qn] = {("1", 0): 0.5, ("c", g): -0.5}
        self.basis += [sn, qn]
        if mat_c:
            self.ops.append(("ts", qn, f"c{g}", -2.0, 1.0))
            self.exp[f"c{g}"] = {("c", g): 1.0}

    # doubling from carriers (cc ~ cos-ish, sc ~ sin-ish): Square(cc), cc*sc
    def dbl(self, cc, sc, sqn, tn):
        self.ops.append(("square", cc, sqn))
        self.exp[sqn] = _expand_mul_fix(self.exp[cc], self.exp[cc])
        self.ops.append(("mul", cc, sc, tn))
        self.exp[tn] = _expand_mul_fix(self.exp[cc], self.exp[sc])
        self.basis += [sqn, tn]

    # quad products of carrier tiles (ca, sa) x (cb, sb)
    def quad(self, ca, sa, cb, sb):
        for (x, y) in ((ca, cb), (sa, sb), (sa, cb), (ca, sb)):
            pn = f"p_{x}_{y}"
            self.ops.append(("mul", x, y, pn))
            self.exp[pn] = _expand_mul_fix(self.exp[x], self.exp[y])
            self.basis.append(pn)


def build_spec():
    sp = Spec()
    sp.direct(1, mat_c=True)
    sp.direct(3, mat_c=True)
    sp.direct(8, mat_c=True)
    sp.direct(12)
    sp.dbl("c1", "s1", "sq2", "t2")      # 2
    sp.dbl("sq2", "t2", "sq4", "t4")     # 4 (affine carriers, basis only)
    sp.dbl("c3", "s3", "sq6", "t6")      # 6
    sp.dbl("c8", "s8", "sq16", "t16")    # 16
    sp.quad("sq6", "t6", "c1", "s1")     # {7,5}
    sp.quad("q12", "s12", "c1", "s1")    # {13,11}
    sp.quad("q12", "s12", "sq2", "t2")   # {14,10}
    sp.quad("q12", "s12", "c3", "s3")    # {15,9}
    sp.exp["one"] = {("1", 0): 1.0}
    sp.basis.insert(0, "one")
    return sp


def _reorder_ops(sp):
    fracs = [o for o in sp.ops if o[0] == "frac"]
    sins = [o for o in sp.ops if o[0] in ("sin2pi", "sinpi")]
    rest = [o for o in sp.ops if o[0] not in ("frac", "sin2pi", "sinpi")]
    sp.ops = fracs + sins + rest
    return sp


SPEC = _reorder_ops(build_spec())
B = len(SPEC.basis)  # 37

HARMONICS = [("1", 0)] + [("c", g) for g in range(1, G + 1)] + [
    ("s", g) for g in range(1, G + 1)
]  # 33


def solve_weights(cos_amp, sin_amp, bias):
    """W[b, i, o] fp64 -> fp16, LN-mean-centered over o."""
    M = np.zeros((B, len(HARMONICS)))
    hidx = {h: k for k, h in enumerate(HARMONICS)}
    for bi, name in enumerate(SPEC.basis):
        for h, v in SPEC.exp[name].items():
            M[bi, hidx[h]] = v
    # target T [33, I, O]
    T = np.zeros((len(HARMONICS), I, O))
    T[0] = bias[None, :] / I
    for g in range(1, G + 1):
        T[hidx[("c", g)]] = cos_amp[:, :, g - 1].T  # [i, o]
        T[hidx[("s", g)]] = sin_amp[:, :, g - 1].T
    piv = np.linalg.pinv(M.T)  # [B, 33]
    resid = np.abs(M.T @ piv - np.eye(len(HARMONICS))).max()
    assert resid < 1e-9, f"basis does not span harmonics: resid={resid}"
    W = np.einsum("bh,hio->bio", piv, T)
    W = W - W.mean(axis=2, keepdims=True)  # center over o (LN mean fold)
    return W


# ---------------------------------------------------------------------------
# Device program
# ---------------------------------------------------------------------------


def build_device_program():
    nc = bass.Bass()
    x_in = nc.declare_dram_parameter("x_sh", [I, NSH], F32, isOutput=False)
    w_in = nc.declare_dram_parameter("w_all", [I, B * O], F16, isOutput=False)
    g_in = nc.declare_dram_parameter("gam", [O, 1], F32, isOutput=False)
    b_in = nc.declare_dram_parameter("bet", [O, 1], F32, isOutput=False)
    out_d = nc.declare_dram_parameter("out_sh", [O, NSH], F32, isOutput=True)

    with ExitStack() as ctx:
        tc = ctx.enter_context(TileContext(nc))
        pool = ctx.enter_context(tc.tile_pool(name="main", bufs=1))
        scr = ctx.enter_context(tc.tile_pool(name="scratch", bufs=2))
        pj = ctx.enter_context(tc.tile_pool(name="psy", bufs=1, space="PSUM"))
        pv = ctx.enter_context(tc.tile_pool(name="psv", bufs=2, space="PSUM"))
        pb = ctx.enter_context(tc.tile_pool(name="psb", bufs=2, space="PSUM"))

        x = pool.tile([I, NSH], F32, tag="x", name="x")
        nc.gpsimd.dma_start(out=x[:], in_=x_in[:])
        wts = pool.tile([I, B * O], F16, tag="wts", name="wts")
        nc.gpsimd.dma_start(out=wts[:], in_=w_in[:])
        gam = pool.tile([O, 1], F32, tag="gam", name="gam")
        bet = pool.tile([O, 1], F32, tag="bet", name="bet")
        nc.gpsimd.dma_start(out=gam[:], in_=g_in[:])
        nc.gpsimd.dma_start(out=bet[:], in_=b_in[:])
        eps_t = pool.tile([1, 1], F32, tag="eps", name="eps")
        nc.vector.memset(eps_t[:], EPS)
        ones_col = pool.tile([I, 1], F16, tag="ones_col", name="ones_col")
        nc.vector.memset(ones_col[:], 1.0)
        ones_row = pool.tile([1, O], F32, tag="ones_row", name="ones_row")
        nc.vector.memset(ones_row[:], 1.0)

        # --- trig basis ---
        tiles = {}

        def tile16(name):
            t = pool.tile([I, NSH], F16, tag="b_" + name, name="b_" + name)
            tiles[name] = t
            return t

        frac = {}
        for op in SPEC.ops:
            if op[0] == "frac":
                g = op[1]
                s = float(np.float32(g / TWO_PI))
                w = scr.tile([I, NSH], F32, tag="fwr", name="fw", bufs=1)
                t = scr.tile([I, NSH], F32, tag="ft", name=f"ft{g}", bufs=2)
                nc.vector.tensor_scalar(w[:], x[:], s, RC, A.mult, A.add)
                nc.vector.tensor_scalar(w[:], w[:], RC, None, A.subtract)
                nc.vector.scalar_tensor_tensor(
                    t[:], x[:], s, w[:], A.mult, A.subtract
                )
                frac[g] = t
            elif op[0] == "sin2pi":
                g, name = op[1], op[2]
                nc.scalar.activation(
                    tile16(name)[:], frac[g][:], AF.Sin, scale=TWO_PI
                )
            elif op[0] == "sinpi":
                g, name = op[1], op[2]
                sh = scr.tile([I, NSH], F16, tag="sh", name="sh", bufs=1)
                tiles[name] = sh
                nc.scalar.activation(sh[:], frac[g][:], AF.Sin, scale=np.pi)
            elif op[0] == "square":
                src_, dst = op[1], op[2]
                nc.scalar.activation(tile16(dst)[:], tiles[src_][:], AF.Square)
            elif op[0] == "ts":
                src_, dst, mul, add = op[1], op[2], op[3], op[4]
                nc.vector.tensor_scalar(
                    tile16(dst)[:], tiles[src_][:], mul, add, A.mult, A.add
                )
            elif op[0] == "mul":
                a_, b_, dst = op[1], op[2], op[3]
                nc.vector.tensor_tensor(
                    tile16(dst)[:], tiles[a_][:], tiles[b_][:], A.mult
                )

        # --- GEMM: y_c[j] = sum_b W_b.T @ basis_b[:, j] ---
        ys = []
        for j in range(NJ):
            ys.append(pj.tile([O, JT], F32, tag=f"y{j}", name=f"y{j}"))
        ones_bc = ones_col[:].to_broadcast((I, JT))

        rstds = {}

        def emit_stats(j):
            sq = scr.tile([O, JT], F16, tag="sq", name="sq")
            nc.scalar.activation(sq[:], ys[j][:], AF.Square)
            vps = pv.tile([1, JT], F32, tag="vps", name="vps")
            nc.tensor.matmul(vps[:], ones_col[:], sq[:], start=True, stop=True)
            var_j = scr.tile([1, JT], F32, tag="var_j", name="var_j")
            nc.scalar.activation(
                var_j[:], vps[:], AF.Ln, scale=1.0 / O, bias=eps_t[:]
            )
            nc.scalar.activation(var_j[:], var_j[:], AF.Exp, scale=-0.5)
            rstds[j] = var_j

        def emit_finalize(j):
            bc = pb.tile([O, JT], F32, tag="bc", name="bc")
            nc.tensor.matmul(
                bc[:], ones_row[:], rstds[j][:], start=True, stop=True
            )
            rb = scr.tile([O, JT], F16, tag="rb", name="rb")
            nc.vector.tensor_copy(rb[:], bc[:])
            t1 = scr.tile([O, JT], F32, tag="t1", name="t1")
            nc.vector.tensor_tensor(t1[:], ys[j][:], rb[:], A.mult)
            oj = scr.tile([O, JT], F32, tag="oj", name="oj")
            nc.vector.tensor_scalar(
                oj[:], t1[:], gam[:], bet[:], A.mult, A.add
            )
            nc.gpsimd.dma_start(
                out=out_d[:, j * JT : (j + 1) * JT], in_=oj[:]
            )

        def emit_mm(j, bi, name):
            rhs = (
                ones_bc
                if name == "one"
                else tiles[name][:, j * JT : (j + 1) * JT]
            )
            nc.tensor.matmul(
                ys[j][:],
                wts[:, bi * O : (bi + 1) * O],
                rhs,
                start=(bi == 0),
                stop=(bi == B - 1),
            )

        # spread phase: j 0..2 consume each basis tile as it lands (PE keeps
        # pace even clock-gated); j=3 deferred into a dense tail stream that
        # self-warms the PE while stats/finalize chains for j 0..2 overlap.
        for bi, name in enumerate(SPEC.basis):
            for j in range(3):
                emit_mm(j, bi, name)
        emit_stats(0)
        third = (B + 2) // 3
        for bi in range(third):
            emit_mm(3, bi, SPEC.basis[bi])
        emit_stats(1)
        emit_finalize(0)
        for bi in range(third, 2 * third):
            emit_mm(3, bi, SPEC.basis[bi])
        emit_stats(2)
        emit_finalize(1)
        for bi in range(2 * third, B):
            emit_mm(3, bi, SPEC.basis[bi])
        emit_finalize(2)
        emit_stats(3)
        emit_finalize(3)
    return nc


_NC_CACHE = None


def kernel(x, cos_amplitudes, sin_amplitudes, bias, ln_gamma, ln_beta):
    global _NC_CACHE
    from concourse.bass_utils import run_bass_kernel_spmd

    x = np.asarray(x, dtype=np.float32)
    ca = np.asarray(cos_amplitudes, dtype=np.float64)
    sa = np.asarray(sin_amplitudes, dtype=np.float64)
    bv = np.asarray(bias, dtype=np.float64)
    gv = np.asarray(ln_gamma, dtype=np.float32).reshape(O, 1)
    be = np.asarray(ln_beta, dtype=np.float32).reshape(O, 1)

    W = solve_weights(ca, sa, bv)  # [B, I, O] fp64 centered
    w_all = np.ascontiguousarray(
        W.transpose(1, 0, 2).reshape(I, B * O)
    ).astype(np.float16)

    xT = np.ascontiguousarray(x.T)  # [I, N]

    if _NC_CACHE is None:
        _NC_CACHE = build_device_program()
    nc = _NC_CACHE

    in_maps = []
    for c in range(NCORES):
        in_maps.append(
            {
                "x_sh": np.ascontiguousarray(xT[:, c * NSH : (c + 1) * NSH]),
                "w_all": w_all,
                "gam": gv,
                "bet": be,
            }
        )
    res = run_bass_kernel_spmd(nc, in_maps, list(range(NCORES)))
    outs = [res.results[c]["out_sh"] for c in range(NCORES)]
    full = np.concatenate(outs, axis=1)  # [O, N]
    return np.ascontiguousarray(full.T).astype(np.float32)

